# revision 4
# baseline (speedup 1.0000x reference)
"""v8: software-pipelined schedule on top of v7's fp16 datapath.

Changes vs v7 (all aimed at keeping the PE queue non-empty and the
ScalarE exp stream hidden under PE work):
 - fast start: weight DMAs issued first, x loaded in 16 chunk-ordered
   pieces split across the two HWDGE queues (sync + scalar) so the
   first qkv matmul starts ~8us in instead of ~37us.
 - k-projections + v-projections run as a prologue; q-projections and
   the next pair's q/k projections are interleaved INTO the attention
   lk-loop (at lk==7 and lk==15) so the in-order PE queue always has
   filler work while ScalarE exp paces the softmax.
 - o-projection for chunk cq is emitted during attention of chunk cq+1,
   so it never waits on the just-finished softmax normalization chain.
 - psum->sbuf copies (v rearrange, o output) moved from ScalarE to DVE;
   softmax reciprocal reads PSUM directly (no staging copy).
"""

import os
import sys

for _p in ("/opt/trn_rl_repo", "/root/.axon_site/_ro/trn_rl_repo"):
    if os.path.isdir(_p) and _p not in sys.path:
        sys.path.insert(0, _p)

import contextlib

import numpy as np

import concourse.bass as bass
import concourse.tile as tile
from concourse import bacc, mybir
from concourse.bass_utils import run_bass_kernel_spmd

P = 128
L = 2048
D = 1536
HL = 6
HD = 64
EQ = 384
NQK = 768
DC = D // P      # 12
LT = L // P      # 16
ACH = 512        # attention lq chunk == qkv l chunk
NCQ = L // ACH   # 4
F32 = mybir.dt.float32
F16 = mybir.dt.float16
AF = mybir.ActivationFunctionType


def build_bass(repeat=1):
    nc = bacc.Bacc("TRN2", target_bir_lowering=False, debug=False, num_devices=8)
    xT = nc.dram_tensor("xT", [D, L], F16, kind="ExternalInput")
    wqkT = nc.dram_tensor("wqkT", [D, NQK], F16, kind="ExternalInput")
    wvT = nc.dram_tensor("wvT", [D, EQ], F16, kind="ExternalInput")
    woT = nc.dram_tensor("woT", [EQ, D], F16, kind="ExternalInput")
    cos2 = nc.dram_tensor("cos2", [P, L], F16, kind="ExternalInput")
    ss2 = nc.dram_tensor("ss2", [P, L], F16, kind="ExternalInput")
    out = nc.dram_tensor("out", [L, D], F32, kind="ExternalOutput")

    xT_r = xT.rearrange("(dc p) l -> p dc l", p=P)
    wqkT_r = wqkT.rearrange("(dc p) e -> p dc e", p=P)
    wvT_r = wvT.rearrange("(dc p) e -> p dc e", p=P)
    woT_r = woT.rearrange("(ec p) d -> p ec d", p=P)

    with tile.TileContext(nc) as tc:
        rep_cm = tc.For_i(0, repeat, 1) if repeat > 1 else contextlib.nullcontext()
        with rep_cm, tc.tile_pool(name="persist", bufs=1) as persist:
            xsb = persist.tile([P, DC, L], F16)
            qT = persist.tile([P, 3, L], F16)
            kT = persist.tile([P, 3, L], F16)
            v1 = persist.tile([P, LT, HL, HD + 1], F16)
            cos_sb = persist.tile([P, L], F16)
            ss_sb = persist.tile([P, L], F16)
            outT = persist.tile([P, 3, L], F16)
            wqks_all = persist.tile([P, DC, 3, 2, P], F16)  # [dc, etp, q/k, 128]
            wv_sb = persist.tile([P, DC, EQ], F16)
            wo_sb = persist.tile([P, 3, D], F16)

            # --- DMA: weights first (needed by the first matmuls), then x
            # pieces in consumption order, alternating the two HWDGE queues.
            nc.scalar.dma_start(cos_sb[:], cos2[:])
            nc.scalar.dma_start(ss_sb[:], ss2[:])
            for d0 in range(0, DC, 3):
                dsl = slice(d0, d0 + 3)
                for etp in range(3):
                    nc.sync.dma_start(
                        wqks_all[:, dsl, etp, 0, :],
                        wqkT_r[:, dsl, etp * P : (etp + 1) * P],
                    )
                    nc.sync.dma_start(
                        wqks_all[:, dsl, etp, 1, :],
                        wqkT_r[:, dsl, EQ + etp * P : EQ + (etp + 1) * P],
                    )
                nc.scalar.dma_start(wv_sb[:, dsl, :], wvT_r[:, dsl, :])
            for c in range(NCQ):
                sl = slice(c * ACH, (c + 1) * ACH)
                for i, d0 in enumerate(range(0, DC, 3)):
                    eng = nc.sync if i % 2 == 0 else nc.scalar
                    eng.dma_start(
                        xsb[:, d0 : d0 + 3, sl], xT_r[:, d0 : d0 + 3, sl]
                    )
            nc.sync.dma_start(wo_sb[:], woT_r[:])

            ones_c = nc.const_aps.tensor(1.0, (P, 1), F32)
            nc.vector.tensor_copy(
                v1[:, :, :, HD : HD + 1], ones_c.to_broadcast([P, LT, HL, 1])
            )

            with (
                tc.tile_pool(name="s2t", bufs=2) as s2t,
                tc.tile_pool(name="s2att", bufs=2) as s2att,
                tc.tile_pool(name="s2o", bufs=3) as s2o,
                tc.tile_pool(name="s2nrm", bufs=3) as s2nrm,
                tc.tile_pool(name="ps_acc", bufs=2, space=bass.MemorySpace.PSUM) as ps_acc,
                tc.tile_pool(name="ps_s", bufs=2, space=bass.MemorySpace.PSUM) as ps_s,
                tc.tile_pool(name="ps_av", bufs=2, space=bass.MemorySpace.PSUM) as ps_av,
            ):

                def qk_group(etp, c, half):
                    """project 512 tokens onto 2 heads' q (half=0) or k
                    (half=1) dims, apply rope, store into qT/kT."""
                    sl = slice(c * ACH, (c + 1) * ACH)
                    ps = ps_acc.tile([P, ACH], F32, tag="acc")
                    for dc in range(DC):
                        nc.tensor.matmul(
                            ps[:],
                            wqks_all[:, dc, etp, half, :],
                            xsb[:, dc, sl],
                            start=(dc == 0),
                            stop=(dc == DC - 1),
                        )
                    dst = (qT if half == 0 else kT)[:, etp, sl]
                    tcos = s2t.tile([P, ACH], F32, tag="tcos")
                    trot = s2t.tile([P, ACH], F32, tag="trot")
                    nc.vector.tensor_mul(tcos[:], ps[:], cos_sb[:, sl])
                    for q_ in range(4):
                        s = (q_ ^ 1) * 32
                        d_ = q_ * 32
                        nc.vector.tensor_mul(
                            trot[d_ : d_ + 32, :],
                            ps[s : s + 32, :],
                            ss_sb[d_ : d_ + 32, sl],
                        )
                    nc.vector.tensor_add(dst, tcos[:], trot[:])

                def v_group(lk):
                    """project one 128-token tile onto all 6 heads' v dims."""
                    pv = ps_acc.tile([P, ACH], F32, tag="acc")
                    for dc in range(DC):
                        nc.tensor.matmul(
                            pv[:, 0:EQ],
                            xsb[:, dc, lk * P : (lk + 1) * P],
                            wv_sb[:, dc, :],
                            start=(dc == 0),
                            stop=(dc == DC - 1),
                        )
                    nc.vector.tensor_copy(
                        v1[:, lk, :, 0:HD],
                        pv[:, 0:EQ].rearrange("p (h d) -> p h d", h=HL),
                    )

                def o_group(cq):
                    """o-projection for one 512-token chunk (all heads)."""
                    for lt in range(ACH // P):
                        l0 = cq * ACH + lt * P
                        for dn in range(D // ACH):
                            pso = ps_acc.tile([P, ACH], F32, tag="acc")
                            for ec in range(3):
                                nc.tensor.matmul(
                                    pso[:],
                                    outT[:, ec, l0 : l0 + P],
                                    wo_sb[:, ec, dn * ACH : (dn + 1) * ACH],
                                    start=(ec == 0),
                                    stop=(ec == 2),
                                )
                            ot = s2o.tile([P, ACH], F32)
                            nc.vector.tensor_copy(ot[:], pso[:])
                            nc.sync.dma_start(
                                out[l0 : l0 + P, dn * ACH : (dn + 1) * ACH],
                                ot[:],
                            )

                def attention_cq(etp, cq, fillers):
                    """attention for heads 2*etp/2*etp+1 over one 512-query
                    chunk; fillers = up to 2 closures emitted at lk 7/15 to
                    keep the PE busy while ScalarE exp paces the loop."""
                    cqs = slice(cq * ACH, (cq + 1) * ACH)
                    pav0 = ps_av.tile([HD + 1, ACH], F32, tag="av")
                    pav1 = ps_av.tile([HD + 1, ACH], F32, tag="av")
                    for lk in range(LT):
                        pscore = ps_s.tile([P, 2 * ACH], F32)
                        att = s2att.tile([P, 2 * ACH], F16)
                        for hh in range(2):  # row-tiled pair, concurrent
                            po = hh * HD
                            nc.tensor.matmul(
                                pscore[:, hh * ACH : (hh + 1) * ACH],
                                kT[po : po + HD, etp, lk * P : (lk + 1) * P],
                                qT[po : po + HD, etp, cqs],
                                start=True,
                                stop=True,
                            )
                        nc.scalar.activation(att[:], pscore[:], AF.Exp, scale=0.125)
                        for hh, pav in ((0, pav0), (1, pav1)):
                            nc.tensor.matmul(
                                pav[:],
                                v1[:, lk, 2 * etp + hh, :],
                                att[:, hh * ACH : (hh + 1) * ACH],
                                start=(lk == 0),
                                stop=(lk == LT - 1),
                            )
                        if lk == 7 and len(fillers) > 0:
                            fillers[0]()
                        if lk == 15 and len(fillers) > 1:
                            fillers[1]()
                    for hh, pav in ((0, pav0), (1, pav1)):
                        po = hh * HD
                        dcp = s2nrm.tile([1, ACH], F32, tag="dcp")
                        nc.vector.tensor_copy(dcp[:], pav[HD : HD + 1, :])
                        rcp = s2nrm.tile([1, ACH], F32, tag="rcp")
                        nc.vector.reciprocal_approx_fast(out=rcp[:], in_=dcp[:])
                        rb = s2nrm.tile([HD, ACH], F32, tag="rb")
                        nc.gpsimd.partition_broadcast(rb[:], rcp[:], channels=HD)
                        nc.vector.tensor_mul(
                            outT[po : po + HD, etp, cqs], pav[0:HD, :], rb[:]
                        )

                # --- prologue: k projections for pair 0, all v projections
                for c in range(NCQ):
                    qk_group(0, c, 1)
                    for lt2 in range(ACH // P):
                        v_group(c * (ACH // P) + lt2)
                qk_group(0, 0, 0)

                # --- pipelined attention: fillers are next-pair projections
                # (q of current pair first — attention cq+1 needs it), with
                # o-projection of chunk cq-1 interleaved during pair 2.
                slots = {
                    0: [
                        [lambda: qk_group(0, 1, 0), lambda: qk_group(1, 0, 1)],
                        [lambda: qk_group(0, 2, 0), lambda: qk_group(1, 1, 1)],
                        [lambda: qk_group(0, 3, 0), lambda: qk_group(1, 2, 1)],
                        [lambda: qk_group(1, 3, 1), lambda: qk_group(1, 0, 0)],
                    ],
                    1: [
                        [lambda: qk_group(1, 1, 0), lambda: qk_group(2, 0, 1)],
                        [lambda: qk_group(1, 2, 0), lambda: qk_group(2, 1, 1)],
                        [lambda: qk_group(1, 3, 0), lambda: qk_group(2, 2, 1)],
                        [lambda: qk_group(2, 3, 1), lambda: qk_group(2, 0, 0)],
                    ],
                    2: [
                        [lambda: qk_group(2, 1, 0)],
                        [lambda: qk_group(2, 2, 0), lambda: o_group(0)],
                        [lambda: qk_group(2, 3, 0), lambda: o_group(1)],
                        [lambda: o_group(2)],
                    ],
                }
                for etp in range(3):
                    for cq in range(NCQ):
                        attention_cq(etp, cq, slots[etp][cq])
                o_group(3)

    nc.compile()
    return nc


_NC_CACHE = None


def _get_nc():
    global _NC_CACHE
    if _NC_CACHE is None:
        _NC_CACHE = build_bass()
    return _NC_CACHE


def make_in_maps(x, w_qkv, w_o, cos, sin):
    x = np.asarray(x, dtype=np.float32)
    w_qkv = np.asarray(w_qkv, dtype=np.float32)
    w_o = np.asarray(w_o, dtype=np.float32)
    cos = np.asarray(cos, dtype=np.float32)
    sin = np.asarray(sin, dtype=np.float32)

    cosT = np.ascontiguousarray(cos.T)
    sinT = sin.T
    ss = np.concatenate([-sinT[0:32], sinT[32:64]], axis=0)
    cos2 = np.ascontiguousarray(np.tile(cosT, (2, 1))).astype(np.float16)
    ss2 = np.ascontiguousarray(np.tile(ss, (2, 1))).astype(np.float16)

    in_maps = []
    for c in range(8):
        b, g = c // 4, c % 4
        xTc = np.ascontiguousarray(x[b].T).astype(np.float16)
        wq = w_qkv[g * EQ : (g + 1) * EQ]
        wk = w_qkv[D + g * EQ : D + (g + 1) * EQ]
        wv = w_qkv[2 * D + g * EQ : 2 * D + (g + 1) * EQ]
        wqkTc = np.ascontiguousarray(np.concatenate([wq, wk], 0).T).astype(np.float16)
        wvTc = np.ascontiguousarray(wv.T).astype(np.float16)
        woTc = np.ascontiguousarray(w_o[:, g * EQ : (g + 1) * EQ].T).astype(np.float16)
        in_maps.append(
            {
                "xT": xTc,
                "wqkT": wqkTc,
                "wvT": wvTc,
                "woT": woTc,
                "cos2": cos2,
                "ss2": ss2,
            }
        )
    return in_maps


def kernel(x, w_qkv, w_o, cos, sin):
    nc = _get_nc()
    in_maps = make_in_maps(x, w_qkv, w_o, cos, sin)
    res = run_bass_kernel_spmd(nc, in_maps, core_ids=list(range(8)))
    outs = [res.results[c]["out"] for c in range(8)]
    full = np.stack(
        [
            outs[0] + outs[1] + outs[2] + outs[3],
            outs[4] + outs[5] + outs[6] + outs[7],
        ]
    ).astype(np.float32)
    return full


# revision 8
# speedup vs baseline: 1.1819x; 1.1819x over previous
"""v8: software-pipelined schedule on top of v7's fp16 datapath.

Changes vs v7 (all aimed at keeping the PE queue non-empty and the
ScalarE exp stream hidden under PE work):
 - fast start: weight DMAs issued first, x loaded in 16 chunk-ordered
   pieces split across the two HWDGE queues (sync + scalar) so the
   first qkv matmul starts ~8us in instead of ~37us.
 - k-projections + v-projections run as a prologue; q-projections and
   the next pair's q/k projections are interleaved INTO the attention
   lk-loop (at lk==7 and lk==15) so the in-order PE queue always has
   filler work while ScalarE exp paces the softmax.
 - o-projection for chunk cq is emitted during attention of chunk cq+1,
   so it never waits on the just-finished softmax normalization chain.
 - psum->sbuf copies (v rearrange, o output) moved from ScalarE to DVE;
   softmax reciprocal reads PSUM directly (no staging copy).
"""

import os
import sys

for _p in ("/opt/trn_rl_repo", "/root/.axon_site/_ro/trn_rl_repo"):
    if os.path.isdir(_p) and _p not in sys.path:
        sys.path.insert(0, _p)

import contextlib

import numpy as np

import concourse.bass as bass
import concourse.tile as tile
from concourse import bacc, mybir
from concourse.bass_utils import run_bass_kernel_spmd

P = 128
L = 2048
D = 1536
HL = 6
HD = 64
EQ = 384
NQK = 768
DC = D // P      # 12
LT = L // P      # 16
ACH = 512        # attention lq chunk == qkv l chunk
NCQ = L // ACH   # 4
F32 = mybir.dt.float32
F16 = mybir.dt.float16
AF = mybir.ActivationFunctionType


def build_bass(repeat=1):
    nc = bacc.Bacc("TRN2", target_bir_lowering=False, debug=False, num_devices=8)
    xT = nc.dram_tensor("xT", [D, L], F16, kind="ExternalInput")
    wqkT = nc.dram_tensor("wqkT", [D, NQK], F16, kind="ExternalInput")
    wvT = nc.dram_tensor("wvT", [D, EQ], F16, kind="ExternalInput")
    woT = nc.dram_tensor("woT", [EQ, D], F16, kind="ExternalInput")
    cos2 = nc.dram_tensor("cos2", [P, L], F16, kind="ExternalInput")
    ss2 = nc.dram_tensor("ss2", [P, L], F16, kind="ExternalInput")
    out = nc.dram_tensor("out", [L, D], F32, kind="ExternalOutput")

    xT_r = xT.rearrange("(dc p) l -> p dc l", p=P)
    wqkT_r = wqkT.rearrange("(dc p) e -> p dc e", p=P)
    wvT_r = wvT.rearrange("(dc p) e -> p dc e", p=P)
    woT_r = woT.rearrange("(ec p) d -> p ec d", p=P)

    with tile.TileContext(nc) as tc:
        rep_cm = tc.For_i(0, repeat, 1) if repeat > 1 else contextlib.nullcontext()
        with rep_cm, tc.tile_pool(name="persist", bufs=1) as persist:
            xsb = persist.tile([P, DC, L], F16)
            qT = persist.tile([P, 3, L], F16)
            kT = persist.tile([P, 3, L], F16)
            v1 = persist.tile([P, LT, HL, HD + 1], F16)
            cos_sb = persist.tile([P, L], F16)
            ss_sb = persist.tile([P, L], F16)
            outT = persist.tile([P, 3, L], F16)
            wqks_all = persist.tile([P, DC, 3, 2, P], F16)  # [dc, etp, q/k, 128]
            wv_sb = persist.tile([P, DC, EQ], F16)
            wo_sb = persist.tile([P, 3, D], F16)

            # --- DMA: weights first (needed by the first matmuls), then x
            # pieces in consumption order, alternating the two HWDGE queues.
            nc.scalar.dma_start(cos_sb[:], cos2[:])
            nc.scalar.dma_start(ss_sb[:], ss2[:])
            # pair-0 k weights + v weights first (prologue needs them), then
            # x pieces in consumption order, then the remaining weights.
            for d0 in range(0, DC, 3):
                dsl = slice(d0, d0 + 3)
                nc.sync.dma_start(
                    wqks_all[:, dsl, 0, 1, :], wqkT_r[:, dsl, EQ : EQ + P]
                )
                nc.scalar.dma_start(wv_sb[:, dsl, :], wvT_r[:, dsl, :])
            for c in range(NCQ):
                sl = slice(c * ACH, (c + 1) * ACH)
                for i, d0 in enumerate(range(0, DC, 3)):
                    eng = nc.sync if i % 2 == 0 else nc.scalar
                    eng.dma_start(
                        xsb[:, d0 : d0 + 3, sl], xT_r[:, d0 : d0 + 3, sl]
                    )
            for d0 in range(0, DC, 3):
                dsl = slice(d0, d0 + 3)
                nc.sync.dma_start(
                    wqks_all[:, dsl, 0, 0, :], wqkT_r[:, dsl, 0:P]
                )
                for etp in range(1, 3):
                    nc.sync.dma_start(
                        wqks_all[:, dsl, etp, 0, :],
                        wqkT_r[:, dsl, etp * P : (etp + 1) * P],
                    )
                    nc.scalar.dma_start(
                        wqks_all[:, dsl, etp, 1, :],
                        wqkT_r[:, dsl, EQ + etp * P : EQ + (etp + 1) * P],
                    )
            nc.sync.dma_start(wo_sb[:], woT_r[:])

            ones_c = nc.const_aps.tensor(1.0, (P, 1), F32)
            nc.vector.tensor_copy(
                v1[:, :, :, HD : HD + 1], ones_c.to_broadcast([P, LT, HL, 1])
            )

            with (
                tc.tile_pool(name="s2t", bufs=2) as s2t,
                tc.tile_pool(name="s2att", bufs=2) as s2att,
                tc.tile_pool(name="s2o", bufs=3) as s2o,
                tc.tile_pool(name="s2nrm", bufs=3) as s2nrm,
                tc.tile_pool(name="ps_acc", bufs=2, space=bass.MemorySpace.PSUM) as ps_acc,
                tc.tile_pool(name="ps_s", bufs=2, space=bass.MemorySpace.PSUM) as ps_s,
                tc.tile_pool(name="ps_av", bufs=2, space=bass.MemorySpace.PSUM) as ps_av,
            ):

                def qk_group(etp, c, half):
                    """project 512 tokens onto 2 heads' q (half=0) or k
                    (half=1) dims, apply rope, store into qT/kT."""
                    sl = slice(c * ACH, (c + 1) * ACH)
                    ps = ps_acc.tile([P, ACH], F32, tag="acc")
                    for dc in range(DC):
                        nc.tensor.matmul(
                            ps[:],
                            wqks_all[:, dc, etp, half, :],
                            xsb[:, dc, sl],
                            start=(dc == 0),
                            stop=(dc == DC - 1),
                        )
                    dst = (qT if half == 0 else kT)[:, etp, sl]
                    tcos = s2t.tile([P, ACH], F32, tag="tcos")
                    trot = s2t.tile([P, ACH], F32, tag="trot")
                    nc.vector.tensor_mul(tcos[:], ps[:], cos_sb[:, sl])
                    for q_ in range(4):
                        s = (q_ ^ 1) * 32
                        d_ = q_ * 32
                        nc.vector.tensor_mul(
                            trot[d_ : d_ + 32, :],
                            ps[s : s + 32, :],
                            ss_sb[d_ : d_ + 32, sl],
                        )
                    nc.vector.tensor_add(dst, tcos[:], trot[:])

                def v_group(lk):
                    """project one 128-token tile onto all 6 heads' v dims."""
                    pv = ps_acc.tile([P, ACH], F32, tag="acc")
                    for dc in range(DC):
                        nc.tensor.matmul(
                            pv[:, 0:EQ],
                            xsb[:, dc, lk * P : (lk + 1) * P],
                            wv_sb[:, dc, :],
                            start=(dc == 0),
                            stop=(dc == DC - 1),
                        )
                    nc.scalar.copy(
                        v1[:, lk, :, 0:HD],
                        pv[:, 0:EQ].rearrange("p (h d) -> p h d", h=HL),
                    )

                def o_group(cq):
                    """o-projection for one 512-token chunk (all heads).
                    psum->sbuf copies alternate ScalarE/DVE to split the
                    load."""
                    for lt in range(ACH // P):
                        l0 = cq * ACH + lt * P
                        for dn in range(D // ACH):
                            pso = ps_acc.tile([P, ACH], F32, tag="acc")
                            for ec in range(3):
                                nc.tensor.matmul(
                                    pso[:],
                                    outT[:, ec, l0 : l0 + P],
                                    wo_sb[:, ec, dn * ACH : (dn + 1) * ACH],
                                    start=(ec == 0),
                                    stop=(ec == 2),
                                )
                            ot = s2o.tile([P, ACH], F32)
                            if dn % 2 == 0:
                                nc.vector.tensor_copy(ot[:], pso[:])
                            else:
                                nc.scalar.copy(ot[:], pso[:])
                            nc.sync.dma_start(
                                out[l0 : l0 + P, dn * ACH : (dn + 1) * ACH],
                                ot[:],
                            )

                def attention_cq(etp, cq, fillers):
                    """attention for heads 2*etp/2*etp+1 over one 512-query
                    chunk; fillers = up to 2 closures emitted at lk 7/15 to
                    keep the PE busy while ScalarE exp paces the loop."""
                    cqs = slice(cq * ACH, (cq + 1) * ACH)
                    pav0 = ps_av.tile([HD + 1, ACH], F32, tag="av")
                    pav1 = ps_av.tile([HD + 1, ACH], F32, tag="av")
                    for lk in range(LT):
                        pscore = ps_s.tile([P, 2 * ACH], F32)
                        att = s2att.tile([P, 2 * ACH], F16)
                        for hh in range(2):  # row-tiled pair, concurrent
                            po = hh * HD
                            nc.tensor.matmul(
                                pscore[:, hh * ACH : (hh + 1) * ACH],
                                kT[po : po + HD, etp, lk * P : (lk + 1) * P],
                                qT[po : po + HD, etp, cqs],
                                start=True,
                                stop=True,
                            )
                        nc.scalar.activation(att[:], pscore[:], AF.Exp, scale=0.125)
                        for hh, pav in ((0, pav0), (1, pav1)):
                            nc.tensor.matmul(
                                pav[:],
                                v1[:, lk, 2 * etp + hh, :],
                                att[:, hh * ACH : (hh + 1) * ACH],
                                start=(lk == 0),
                                stop=(lk == LT - 1),
                            )
                        if lk == 6 and len(fillers) > 0:
                            fillers[0]()
                        if lk == 12 and len(fillers) > 1:
                            fillers[1]()
                    for hh, pav in ((0, pav0), (1, pav1)):
                        po = hh * HD
                        dcp = s2nrm.tile([1, ACH], F32, tag="dcp")
                        nc.vector.tensor_copy(dcp[:], pav[HD : HD + 1, :])
                        rcp = s2nrm.tile([1, ACH], F32, tag="rcp")
                        nc.vector.reciprocal_approx_fast(out=rcp[:], in_=dcp[:])
                        rb = s2nrm.tile([HD, ACH], F32, tag="rb")
                        nc.gpsimd.partition_broadcast(rb[:], rcp[:], channels=HD)
                        nc.vector.tensor_mul(
                            outT[po : po + HD, etp, cqs], pav[0:HD, :], rb[:]
                        )

                # --- prologue: k projections for pair 0, all v projections
                for c in range(NCQ):
                    qk_group(0, c, 1)
                    for lt2 in range(ACH // P):
                        v_group(c * (ACH // P) + lt2)
                qk_group(0, 0, 0)

                # --- pipelined attention: fillers are next-pair projections
                # (q of current pair first — attention cq+1 needs it), with
                # o-projection of chunk cq-1 interleaved during pair 2.
                slots = {
                    0: [
                        [lambda: qk_group(0, 1, 0), lambda: qk_group(1, 0, 1)],
                        [lambda: qk_group(0, 2, 0), lambda: qk_group(1, 1, 1)],
                        [lambda: qk_group(0, 3, 0), lambda: qk_group(1, 2, 1)],
                        [lambda: qk_group(1, 3, 1), lambda: qk_group(1, 0, 0)],
                    ],
                    1: [
                        [lambda: qk_group(1, 1, 0), lambda: qk_group(2, 0, 1)],
                        [lambda: qk_group(1, 2, 0), lambda: qk_group(2, 1, 1)],
                        [lambda: qk_group(1, 3, 0), lambda: qk_group(2, 2, 1)],
                        [lambda: qk_group(2, 3, 1), lambda: qk_group(2, 0, 0)],
                    ],
                    2: [
                        [lambda: qk_group(2, 1, 0)],
                        [lambda: qk_group(2, 2, 0), lambda: o_group(0)],
                        [lambda: qk_group(2, 3, 0), lambda: o_group(1)],
                        [lambda: o_group(2)],
                    ],
                }
                for etp in range(3):
                    for cq in range(NCQ):
                        attention_cq(etp, cq, slots[etp][cq])
                o_group(3)

    nc.compile()
    return nc


_NC_CACHE = None


def _get_nc():
    global _NC_CACHE
    if _NC_CACHE is None:
        _NC_CACHE = build_bass()
    return _NC_CACHE


def make_in_maps(x, w_qkv, w_o, cos, sin):
    x = np.asarray(x, dtype=np.float32)
    w_qkv = np.asarray(w_qkv, dtype=np.float32)
    w_o = np.asarray(w_o, dtype=np.float32)
    cos = np.asarray(cos, dtype=np.float32)
    sin = np.asarray(sin, dtype=np.float32)

    cosT = np.ascontiguousarray(cos.T)
    sinT = sin.T
    ss = np.concatenate([-sinT[0:32], sinT[32:64]], axis=0)
    cos2 = np.ascontiguousarray(np.tile(cosT, (2, 1))).astype(np.float16)
    ss2 = np.ascontiguousarray(np.tile(ss, (2, 1))).astype(np.float16)

    in_maps = []
    for c in range(8):
        b, g = c // 4, c % 4
        xTc = np.ascontiguousarray(x[b].T).astype(np.float16)
        wq = w_qkv[g * EQ : (g + 1) * EQ]
        wk = w_qkv[D + g * EQ : D + (g + 1) * EQ]
        wv = w_qkv[2 * D + g * EQ : 2 * D + (g + 1) * EQ]
        wqkTc = np.ascontiguousarray(np.concatenate([wq, wk], 0).T).astype(np.float16)
        wvTc = np.ascontiguousarray(wv.T).astype(np.float16)
        woTc = np.ascontiguousarray(w_o[:, g * EQ : (g + 1) * EQ].T).astype(np.float16)
        in_maps.append(
            {
                "xT": xTc,
                "wqkT": wqkTc,
                "wvT": wvTc,
                "woT": woTc,
                "cos2": cos2,
                "ss2": ss2,
            }
        )
    return in_maps


def kernel(x, w_qkv, w_o, cos, sin):
    nc = _get_nc()
    in_maps = make_in_maps(x, w_qkv, w_o, cos, sin)
    res = run_bass_kernel_spmd(nc, in_maps, core_ids=list(range(8)))
    outs = [res.results[c]["out"] for c in range(8)]
    full = np.stack(
        [
            outs[0] + outs[1] + outs[2] + outs[3],
            outs[4] + outs[5] + outs[6] + outs[7],
        ]
    ).astype(np.float32)
    return full
